# revision 1
# baseline (speedup 1.0000x reference)
"""Trainium2 Bass kernel for a 2-layer TransformerConv GNN + MLP head.

Contract: kernel(**inputs) takes the FULL inputs (as produced by
setup_inputs()) and returns the FULL [N, 2] output, running the compute
on 8 NeuronCores via run_bass_kernel_spmd.

Sharding: nodes are padded to 50176 = 8 * 49 * 128 and split into 8
contiguous ranges of 49 node-tiles (128 nodes each). Each core owns the
edges whose *target* (dst) falls in its range (edge/data parallel with
disjoint segment sums -> no all-reduce needed). K/V node projections are
computed shard-wise and all-gathered so every core can gather arbitrary
source rows.

Edge pipeline per 128-node tile: per-edge rows of Q (by dst) and K|V
(by src) are fetched with dma_gather (int16 indices, tables split in two
halves to fit the int16 range); edge-attr projections e = ea @ We are
computed on the TensorEngine from host-transposed edge attributes; the
attention softmax is computed without max-subtraction (mathematically
identical, exp cannot overflow fp32 at these magnitudes); segment sums
over edges are one-hot matmuls into PSUM.
"""

import sys

sys.path.insert(0, "/opt/trn_rl_repo")

import os

import numpy as np
import ml_dtypes

import concourse.bacc as bacc
import concourse.bass as bass
import concourse.mybir as mybir
import concourse.tile as tile
from concourse.bass_utils import run_bass_kernel_spmd
from concourse.masks import make_identity

P = 128
NCORES = 8
FP = mybir.dt.float32

# problem dims (hardcoded per contract)
N_NODES = 50000
N_EDGES = 800000
F_NODE = 128
F_EDGE = 32
HEADS = 4
C1 = 32
C2 = 16
N_CLASSES = 2


# ----------------------------------------------------------------------------
# host-side preprocessing
# ----------------------------------------------------------------------------

def _wrap_idx(a):
    """[T, S] int16 -> dma_gather wrapped layout [T, 128, S//16]:
    index i of a call lands at [i % 16, i // 16], replicated x8 down
    the partitions (each GPSIMD core reads its own 16-partition group)."""
    T, S = a.shape
    w = np.ascontiguousarray(a.reshape(T, S // 16, 16).transpose(0, 2, 1))
    return np.tile(w, (1, 8, 1))


def host_prep(x, edge_index, edge_attr, n_nodes, n_edges, fe):
    """Build per-core device inputs for the edge phase."""
    t_total = -(-n_nodes // P)                      # ceil
    t_core = -(-t_total // NCORES)
    t_all = t_core * NCORES
    n_pad = t_all * P
    n_core = t_core * P
    half = (n_pad // 2 + P - 1) // P * P            # split point for int16 tables
    assert half < 32768 and n_pad - half < 32768

    src = np.asarray(edge_index[0], dtype=np.int64)
    dst = np.asarray(edge_index[1], dtype=np.int64)
    ea = np.asarray(edge_attr, dtype=np.float32)

    tile_of = dst // P
    key = (tile_of * 2 + (src >= half)).astype(np.int64)
    order = np.argsort(key, kind="stable")
    counts = np.bincount(key, minlength=t_all * 2)
    cl = int(-(-counts[0::2].max() // P))           # lo chunks per tile
    ch = int(-(-counts[1::2].max() // P))           # hi chunks per tile
    ct = cl + ch
    cap = ct * P

    sorted_keys = key[order]
    grp_starts = np.concatenate(([0], np.cumsum(counts)[:-1]))
    pos = np.arange(n_edges) - grp_starts[sorted_keys]
    dest = (sorted_keys // 2) * cap + (sorted_keys % 2) * (cl * P) + pos

    slot_edge = np.full(t_all * cap, -1, np.int64)
    slot_edge[dest] = order
    valid = slot_edge >= 0
    e_idx = np.where(valid, slot_edge, 0)
    src_s = src[e_idx]
    dst_s = dst[e_idx]
    t_arr = np.repeat(np.arange(t_all), cap)

    kvidx = np.where(valid, np.where(src_s < half, src_s, src_s - half), 0)
    kvidx = kvidx.astype(np.int16).reshape(t_all, cap)
    core_base = (t_arr // t_core) * n_core
    qidx = np.where(valid, dst_s - core_base, 0).astype(np.int16).reshape(t_all, cap)
    dstrel = np.where(valid, dst_s - t_arr * P, -1).astype(ml_dtypes.bfloat16)
    dstrel = dstrel.reshape(t_all, ct, P)            # [T, chunk, edge-in-chunk]
    ea_slots = np.where(valid[:, None], ea[e_idx], 0).astype(np.float32)
    eaT = np.ascontiguousarray(
        ea_slots.reshape(t_all, cap, fe).transpose(0, 2, 1)
    )                                               # [T, FE, cap]

    # per-section wrapped gather indices, concatenated: [T, 128, ct*8]
    kvw = np.concatenate(
        [_wrap_idx(kvidx[:, : cl * P]), _wrap_idx(kvidx[:, cl * P:])], axis=2
    )
    qw = np.concatenate(
        [_wrap_idx(qidx[:, : cl * P]), _wrap_idx(qidx[:, cl * P:])], axis=2
    )
    # dstrel laid out [T, 128, ct] (partition = edge-in-chunk)
    dstrel_t = np.ascontiguousarray(dstrel.transpose(0, 2, 1))

    x_pad = np.zeros((n_pad, x.shape[1]), np.float32)
    x_pad[:n_nodes] = x

    percore = []
    for c in range(NCORES):
        ts = slice(c * t_core, (c + 1) * t_core)
        percore.append(
            dict(
                xT=np.ascontiguousarray(x_pad[c * n_core:(c + 1) * n_core].T),
                eaT=np.ascontiguousarray(eaT[ts]),
                kvidx=np.ascontiguousarray(
                    kvw[ts].transpose(1, 0, 2).reshape(P, -1)),
                qidx=np.ascontiguousarray(
                    qw[ts].transpose(1, 0, 2).reshape(P, -1)),
                dstrel=np.ascontiguousarray(
                    dstrel_t[ts].transpose(1, 0, 2).reshape(P, -1)),
            )
        )
    dcfg = dict(
        t_core=t_core, cl=cl, ch=ch, half=half, n_pad=n_pad, n_core=n_core,
        fn=x.shape[1], fe=fe, h=HEADS, c1=C1, c2=C2, ncls=N_CLASSES,
    )
    return percore, dcfg


# ----------------------------------------------------------------------------
# device program
# ----------------------------------------------------------------------------

def _edge_layer(nc, tc, pool, psum, cfg, consts, layer):
    """One TransformerConv edge pass over this core's tiles.

    Gathers per-edge Q (by dst) and K|V (by src) rows, computes the edge
    softmax without max-subtraction, and accumulates one-hot segment-sum
    matmuls into PSUM. Epilogues are batched over TG-tile groups; the
    relu'd per-node result lands in layer["h_res"] ([128, t_core*c]).
    """
    t_core, cl, ch = cfg["t_core"], cfg["cl"], cfg["ch"]
    ct = cl + ch
    half, fe, H = cfg["half"], cfg["fe"], cfg["h"]
    c = layer["c"]
    hc = H * c
    iota = consts["iota"]
    kvidx_sb, qidx_sb, dstrel_sb = consts["kvidx"], consts["qidx"], consts["dstrel"]
    scale = 1.0 / float(np.sqrt(c))

    q_dram, kv_full = layer["q_dram"], layer["kv_full"]
    q_step = layer["q_step"]
    We_sb = layer["We_sb"]
    h_res = layer["h_res"]
    G = 6                                            # chunks per DVE slab group
    groups = [(g, min(G, ct - g)) for g in range(0, ct, G)]
    TG = 8                                           # tiles per epilogue batch
    MAXC = 8                     # dma_gather tops out at 1024 indices/call

    # skip connection rows for all own tiles, resident: [128, t_core*c]
    skip_all = layer["pool1"].tile([P, t_core * c], FP, tag="skip_all")
    nc.scalar.dma_start(
        out=skip_all[:].rearrange("p (t w) -> p t w", t=t_core),
        in_=q_dram[:, hc:hc + c].rearrange("(t p) w -> p t w", p=P))

    agg_grp = None
    for t in range(t_core):
        deng = nc.sync if t % 2 == 0 else nc.scalar
        eaT_t = pool.tile([fe, ct * P], FP, tag="eaT")
        deng.dma_start(out=eaT_t[:], in_=layer["eaT_dram"][t])

        q_e = pool.tile([P, ct, hc], FP, tag="q_e")
        kv_e = pool.tile([P, ct, 2 * hc], FP, tag="kv_e")
        if t < 2:
            nc.vector.memset(q_e[:], 0.0)
            nc.vector.memset(kv_e[:], 0.0)
        qi = qidx_sb[:, t * ct * 8:(t + 1) * ct * 8]
        ki = kvidx_sb[:, t * ct * 8:(t + 1) * ct * 8]

        def emit_gathers(out_tile, table_ap, idx_ap, c0, nch, elem, step=None,
                         queue=0):
            for s0 in range(0, nch, MAXC):
                n = min(MAXC, nch - s0)
                nc.gpsimd.dma_gather(
                    out_tile[:, c0 + s0:c0 + s0 + n, :], table_ap,
                    idx_ap[:, (c0 + s0) * 8:(c0 + s0 + n) * 8],
                    n * P, n * P, elem, elem_step=step, queue_num=queue)

        nq = int(os.environ.get("KBUILD_NQ", "4"))
        emit_gathers(q_e, q_dram[:, 0:hc], qi, 0, cl, hc, q_step,
                     queue=1 % nq)
        emit_gathers(q_e, q_dram[:, 0:hc], qi, cl, ch, hc, q_step,
                     queue=3 % nq)
        emit_gathers(kv_e, kv_full[:half, :], ki, 0, cl, 2 * hc, queue=0)
        emit_gathers(kv_e, kv_full[half:, :], ki, cl, ch, 2 * hc,
                     queue=2 % nq)

        agg_ps = psum.tile([P, H * (c + 1)], FP, space="PSUM", tag="agg")
        first = True
        for g0, gn in groups:
            e_ps = psum.tile([P, G * hc], FP, space="PSUM", tag="e_ps")
            for j in range(gn):
                nc.tensor.matmul(
                    out=e_ps[:, j * hc:(j + 1) * hc],
                    lhsT=eaT_t[:, (g0 + j) * P:(g0 + j + 1) * P],
                    rhs=We_sb[:],
                    start=True, stop=True,
                )
            e_v = e_ps[:].rearrange("p (g f) -> p g f", g=G)[:, 0:gn, :]
            ke = pool.tile([P, G * hc], FP, tag="ke")
            ve = pool.tile([P, G * hc], mybir.dt.bfloat16, tag="ve")
            nc.vector.tensor_tensor(
                out=ke[:].rearrange("p (g f) -> p g f", g=G)[:, 0:gn, :],
                in0=kv_e[:, g0:g0 + gn, 0:hc], in1=e_v, op=mybir.AluOpType.add)
            nc.vector.tensor_tensor(
                out=ve[:].rearrange("p (g f) -> p g f", g=G)[:, 0:gn, :],
                in0=kv_e[:, g0:g0 + gn, hc:2 * hc], in1=e_v,
                op=mybir.AluOpType.add)
            nc.vector.tensor_tensor(
                out=ke[:].rearrange("p (g f) -> p g f", g=G)[:, 0:gn, :],
                in0=q_e[:, g0:g0 + gn, :],
                in1=ke[:].rearrange("p (g f) -> p g f", g=G)[:, 0:gn, :],
                op=mybir.AluOpType.mult)
            lg = pool.tile([P, G * H], FP, tag="lg")
            nc.vector.reduce_sum(
                out=lg[:].rearrange("p (g h) -> p g h", g=G)[:, 0:gn, :],
                in_=ke[:].rearrange("p (g h w) -> p g h w", g=G, h=H)[:, 0:gn],
                axis=mybir.AxisListType.X)
            p_t = pool.tile([P, G * H], mybir.dt.bfloat16, tag="p_t")
            nc.scalar.activation(
                out=p_t[:, 0:gn * H], in_=lg[:, 0:gn * H],
                func=mybir.ActivationFunctionType.Exp, scale=scale)
            pv = pool.tile([P, G * H * (c + 1)], mybir.dt.bfloat16, tag="pv")
            pv4 = pv[:].rearrange("p (g h w) -> p g h w", g=G, h=H)
            p3 = p_t[:].rearrange("p (g h) -> p g h", g=G)
            nc.vector.tensor_tensor(
                out=pv4[:, 0:gn, :, 0:c],
                in0=ve[:].rearrange("p (g h w) -> p g h w", g=G, h=H)[:, 0:gn],
                in1=p3[:, 0:gn, :, None].to_broadcast([P, gn, H, c]),
                op=mybir.AluOpType.mult)
            nc.vector.tensor_copy(out=pv4[:, 0:gn, :, c], in_=p3[:, 0:gn, :])
            oh = pool.tile([P, G * P], mybir.dt.bfloat16, tag="oh")
            nc.vector.tensor_tensor(
                out=oh[:].rearrange("p (g f) -> p g f", g=G)[:, 0:gn, :],
                in0=iota[:].rearrange("p (g f) -> p g f", g=G)[:, 0:gn, :],
                in1=dstrel_sb[:, t * ct + g0: t * ct + g0 + gn][:, :, None]
                    .to_broadcast([P, gn, P]),
                op=mybir.AluOpType.is_equal)
            for j in range(gn):
                nc.tensor.matmul(
                    out=agg_ps[:],
                    lhsT=oh[:, j * P:(j + 1) * P],
                    rhs=pv[:, j * H * (c + 1):(j + 1) * H * (c + 1)],
                    start=first, stop=(g0 + j == ct - 1),
                )
                first = False

        # stash this tile's PSUM aggregate; epilogues run batched per TG tiles
        tg = t % TG
        if tg == 0:
            agg_grp = pool.tile([P, TG * H * (c + 1)], FP, tag="agg_grp")
        nc.vector.tensor_copy(
            out=agg_grp[:, tg * H * (c + 1):(tg + 1) * H * (c + 1)],
            in_=agg_ps[:])
        if tg == TG - 1 or t == t_core - 1:
            n = tg + 1
            t0 = t - tg
            a4 = agg_grp[:].rearrange("p (t h w) -> p t h w", t=TG, h=H)
            sp = pool.tile([P, TG * H], FP, tag="sp")
            nc.vector.tensor_scalar(
                out=sp[:, 0:n * H],
                in0=a4[:, 0:n, :, c].rearrange("p t h -> p (t h)"),
                scalar1=1e-30, scalar2=None, op0=mybir.AluOpType.add)
            rs = pool.tile([P, TG * H], FP, tag="rs")
            nc.vector.reciprocal(out=rs[:, 0:n * H], in_=sp[:, 0:n * H])
            nc.vector.tensor_scalar(
                out=rs[:, 0:n * H], in0=rs[:, 0:n * H], scalar1=1.0 / H,
                scalar2=None, op0=mybir.AluOpType.mult)
            nc.vector.tensor_tensor(
                out=a4[:, 0:n, :, 0:c], in0=a4[:, 0:n, :, 0:c],
                in1=rs[:].rearrange("p (t h) -> p t h", t=TG)[:, 0:n, :, None]
                    .to_broadcast([P, n, H, c]),
                op=mybir.AluOpType.mult)
            hsum = pool.tile([P, TG * c], FP, tag="hsum")
            nc.vector.reduce_sum(
                out=hsum[:].rearrange("p (t w) -> p t w", t=TG)[:, 0:n],
                in_=agg_grp[:].rearrange("p (t h w) -> p t w h", t=TG,
                                         h=H)[:, 0:n, 0:c, :],
                axis=mybir.AxisListType.X)
            nc.vector.tensor_tensor(
                out=hsum[:, 0:n * c], in0=hsum[:, 0:n * c],
                in1=skip_all[:, t0 * c:(t0 + n) * c],
                op=mybir.AluOpType.add)
            nc.scalar.activation(
                out=h_res[:, t0 * c:(t0 + n) * c], in_=hsum[:, 0:n * c],
                func=mybir.ActivationFunctionType.Relu)


def build_device(dcfg):
    phases = os.environ.get("KBUILD_PHASES", "F")
    t_core, cl, ch = dcfg["t_core"], dcfg["cl"], dcfg["ch"]
    ct = cl + ch
    n_pad, n_core = dcfg["n_pad"], dcfg["n_core"]
    fn, fe, H = dcfg["fn"], dcfg["fe"], dcfg["h"]
    c1, c2, ncls = dcfg["c1"], dcfg["c2"], dcfg["ncls"]
    hc1, hc2 = H * c1, H * c2
    hid = 2 * c2

    nc = bacc.Bacc("TRN2", target_bir_lowering=False, debug=False,
                   num_devices=NCORES, num_swdge_queues=4)

    def param(name, shape, dtype=FP, out=False):
        return nc.declare_dram_parameter(name, list(shape), dtype, isOutput=out)

    xT_d = param("xT", [fn, n_core])
    eaT_d = param("eaT", [t_core, fe, ct * P])
    kvidx_d = param("kvidx", [P, t_core * ct * 8], mybir.dt.int16)
    qidx_d = param("qidx", [P, t_core * ct * 8], mybir.dt.int16)
    dstrel_d = param("dstrel", [P, t_core * ct], mybir.dt.bfloat16)
    wkv1_d = param("wkv1", [fn, 2 * hc1])
    bkv1_d = param("bkv1", [1, 2 * hc1])
    wqs1_d = param("wqs1", [fn, hc1 + c1])
    bqs1_d = param("bqs1", [1, hc1 + c1])
    we1_d = param("we1", [fe, hc1])
    wkv2_d = param("wkv2", [c1, 2 * hc2])
    bkv2_d = param("bkv2", [1, 2 * hc2])
    wqs2_d = param("wqs2", [c1, hc2 + c2])
    bqs2_d = param("bqs2", [1, hc2 + c2])
    we2_d = param("we2", [fe, hc2])
    w3_d = param("w3", [c2, hid])
    b3_d = param("b3", [hid, 1])
    w4_d = param("w4", [hid, ncls])
    b4_d = param("b4", [ncls, 1])
    out_d = param("out", [ncls, n_core], out=True)

    with tile.TileContext(nc) as tc:
        with (
            tc.tile_pool(name="res", bufs=1) as res,
            tc.tile_pool(name="sbuf", bufs=2) as pool,
            tc.tile_pool(name="sbuf1", bufs=1) as pool1,
            tc.tile_pool(name="dram", bufs=1, space="DRAM") as dram,
        ):
            # ---- constants / resident tensors
            ident = res.tile([P, P], FP)
            make_identity(nc, ident[:])
            ones_row = res.tile([1, P], FP)
            nc.vector.memset(ones_row[:], 1.0)
            iota = res.tile([P, 6 * P], mybir.dt.bfloat16)
            nc.gpsimd.iota(iota[:, 0:P], pattern=[[1, P]], base=0,
                           channel_multiplier=0,
                           allow_small_or_imprecise_dtypes=True)
            for g in range(1, 6):
                nc.vector.tensor_copy(out=iota[:, g * P:(g + 1) * P],
                                      in_=iota[:, 0:P])
            kvidx_sb = res.tile([P, t_core * ct * 8], mybir.dt.int16)
            nc.sync.dma_start(out=kvidx_sb[:], in_=kvidx_d[:])
            qidx_sb = res.tile([P, t_core * ct * 8], mybir.dt.int16)
            nc.sync.dma_start(out=qidx_sb[:], in_=qidx_d[:])
            dstrel_sb = res.tile([P, t_core * ct], mybir.dt.bfloat16)
            nc.sync.dma_start(out=dstrel_sb[:], in_=dstrel_d[:])

            def load_w(d, shape, tag, dt=FP):
                t = res.tile(list(shape), dt, tag=tag)
                nc.sync.dma_start(out=t[:], in_=d[:])
                return t

            wkv1 = load_w(wkv1_d, [fn, 2 * hc1], "wkv1")
            bkv1 = load_w(bkv1_d, [1, 2 * hc1], "bkv1")
            wqs1 = load_w(wqs1_d, [fn, hc1 + c1], "wqs1")
            bqs1 = load_w(bqs1_d, [1, hc1 + c1], "bqs1")
            we1 = load_w(we1_d, [fe, hc1], "we1")
            wkv2 = load_w(wkv2_d, [c1, 2 * hc2], "wkv2")
            bkv2 = load_w(bkv2_d, [1, 2 * hc2], "bkv2")
            wqs2 = load_w(wqs2_d, [c1, hc2 + c2], "wqs2")
            bqs2 = load_w(bqs2_d, [1, hc2 + c2], "bqs2")
            we2 = load_w(we2_d, [fe, hc2], "we2")
            w3 = load_w(w3_d, [c2, hid], "w3")
            b3 = load_w(b3_d, [hid, 1], "b3")
            w4 = load_w(w4_d, [hid, ncls], "w4")
            b4 = load_w(b4_d, [ncls, 1], "b4")

            h1_res = res.tile([P, t_core * c1], FP)
            h2_res = res.tile([P, t_core * c2], FP)
            h2T_res = res.tile([c2, t_core * P], FP)

            # ---- internal DRAM
            kv1_shard = dram.tile([n_core, 2 * hc1], FP)
            kv1_full = dram.tile([n_pad, 2 * hc1], FP)
            qs1_dram = dram.tile([n_core, 192], FP)
            kv2_shard = dram.tile([n_core, 2 * hc2], FP)
            kv2_full = dram.tile([n_pad, 2 * hc2], FP)
            qs2_dram = dram.tile([n_core, 128], FP)

            reps = int(os.environ.get("KBUILD_REPS", "1"))

            def emit_pipeline():
                # ---- phase A: layer-1 projections for own node range
                with tc.tile_pool(name="psumA", bufs=2, space="PSUM") as psum:
                  for t in range(t_core):
                      deng = nc.sync if t % 2 == 0 else nc.scalar
                      xT_t = pool.tile([fn, P], FP, tag="xT_t")
                      deng.dma_start(out=xT_t[:], in_=xT_d[:, t * P:(t + 1) * P])
                      pr_ps = psum.tile([P, 2 * hc1 + hc1 + c1], FP, space="PSUM",
                                        tag="pr_ps")
                      nc.tensor.matmul(out=pr_ps[:, 0:2 * hc1], lhsT=xT_t[:],
                                       rhs=wkv1[:], start=True, stop=False)
                      nc.tensor.matmul(out=pr_ps[:, 0:2 * hc1], lhsT=ones_row[:1, :],
                                       rhs=bkv1[:1, :], start=False, stop=True)
                      nc.tensor.matmul(out=pr_ps[:, 2 * hc1:], lhsT=xT_t[:],
                                       rhs=wqs1[:], start=True, stop=False)
                      nc.tensor.matmul(out=pr_ps[:, 2 * hc1:], lhsT=ones_row[:1, :],
                                       rhs=bqs1[:1, :], start=False, stop=True)
                      pr_sb = pool.tile([P, 2 * hc1 + hc1 + c1], FP, tag="pr_sb")
                      nc.vector.tensor_copy(out=pr_sb[:], in_=pr_ps[:])
                      deng.dma_start(out=kv1_shard[t * P:(t + 1) * P, :],
                                     in_=pr_sb[:, 0:2 * hc1])
                      deng.dma_start(out=qs1_dram[t * P:(t + 1) * P, 0:hc1 + c1],
                                     in_=pr_sb[:, 2 * hc1:])

                if phases >= "AG":
                    nc.gpsimd.collective_compute(
                        "AllGather", mybir.AluOpType.bypass,
                        replica_groups=[list(range(NCORES))],
                        ins=[kv1_shard[:].opt()], outs=[kv1_full[:].opt()])

                consts = dict(iota=iota, kvidx=kvidx_sb, qidx=qidx_sb,
                              dstrel=dstrel_sb)

                # ---- phase B: layer-1 edge pass
                if phases < "B":
                    nc.vector.memset(h1_res[:], 0.0)

                if phases >= "B":
                  with tc.tile_pool(name="psumB", bufs=2, space="PSUM") as psum:
                    _edge_layer(nc, tc, pool, psum, dcfg, consts, dict(
                        c=c1, q_dram=qs1_dram, q_step=192, kv_full=kv1_full,
                        We_sb=we1, eaT_dram=eaT_d,
                        h_res=h1_res[:], pool1=pool1))

                # ---- phase D: layer-2 projections from h1 (own range, resident)
                if phases >= "D":
                 with tc.tile_pool(name="psumD", bufs=2, space="PSUM") as psum:
                  for t in range(t_core):
                      h1T_ps = psum.tile([c1, P], FP, space="PSUM", tag="h1T_ps")
                      nc.tensor.transpose(
                          out=h1T_ps[:], in_=h1_res[:, t * c1:(t + 1) * c1],
                          identity=ident[:])
                      h1T = pool.tile([c1, P], FP, tag="h1T")
                      nc.vector.tensor_copy(out=h1T[:], in_=h1T_ps[:])
                      p2_ps = psum.tile([P, 2 * hc2 + hc2 + c2], FP, space="PSUM",
                                        tag="p2_ps")
                      nc.tensor.matmul(out=p2_ps[:, 0:2 * hc2], lhsT=h1T[:],
                                       rhs=wkv2[:], start=True, stop=False)
                      nc.tensor.matmul(out=p2_ps[:, 0:2 * hc2], lhsT=ones_row[:1, :],
                                       rhs=bkv2[:1, :], start=False, stop=True)
                      nc.tensor.matmul(out=p2_ps[:, 2 * hc2:], lhsT=h1T[:],
                                       rhs=wqs2[:], start=True, stop=False)
                      nc.tensor.matmul(out=p2_ps[:, 2 * hc2:], lhsT=ones_row[:1, :],
                                       rhs=bqs2[:1, :], start=False, stop=True)
                      p2_sb = pool.tile([P, 2 * hc2 + hc2 + c2], FP, tag="p2_sb")
                      nc.vector.tensor_copy(out=p2_sb[:], in_=p2_ps[:])
                      deng = nc.sync if t % 2 == 0 else nc.scalar
                      deng.dma_start(out=kv2_shard[t * P:(t + 1) * P, :],
                                     in_=p2_sb[:, 0:2 * hc2])
                      deng.dma_start(out=qs2_dram[t * P:(t + 1) * P, 0:hc2 + c2],
                                     in_=p2_sb[:, 2 * hc2:])

                if phases >= "D":
                    nc.gpsimd.collective_compute(
                        "AllGather", mybir.AluOpType.bypass,
                        replica_groups=[list(range(NCORES))],
                        ins=[kv2_shard[:].opt()], outs=[kv2_full[:].opt()])

                # ---- phase E: layer-2 edge pass
                if phases >= "E":
                  with tc.tile_pool(name="psumE", bufs=2, space="PSUM") as psum:
                    _edge_layer(nc, tc, pool, psum, dcfg, consts, dict(
                        c=c2, q_dram=qs2_dram, q_step=128, kv_full=kv2_full,
                        We_sb=we2, eaT_dram=eaT_d,
                        h_res=h2_res[:], pool1=pool1))
                    for t in range(t_core):
                        h2T_ps = psum.tile([c2, P], FP, space="PSUM",
                                           tag="h2T_ps")
                        nc.tensor.transpose(
                            out=h2T_ps[:], in_=h2_res[:, t * c2:(t + 1) * c2],
                            identity=ident[:])
                        nc.vector.tensor_copy(
                            out=h2T_res[:, t * P:(t + 1) * P], in_=h2T_ps[:])

                # ---- phase F: dense head on h2T (outputs transposed [ncls, n_core])
                CHUNK = 512
                if phases < "E":
                    nc.vector.memset(h2T_res[:], 0.0)
                with tc.tile_pool(name="psumF", bufs=2, space="PSUM") as psum:
                  for k0 in range(0, n_core, CHUNK):
                      kn = min(CHUNK, n_core - k0)
                      h3_ps = psum.tile([hid, CHUNK], FP, space="PSUM", tag="h3_ps")
                      nc.tensor.matmul(out=h3_ps[:, 0:kn], lhsT=w3[:],
                                       rhs=h2T_res[:, k0:k0 + kn], start=True,
                                       stop=True)
                      h3_sb = pool.tile([hid, CHUNK], FP, tag="h3_sb")
                      nc.scalar.activation(
                          out=h3_sb[:, 0:kn], in_=h3_ps[:, 0:kn],
                          func=mybir.ActivationFunctionType.Relu, bias=b3[:, 0:1])
                      o_ps = psum.tile([ncls, CHUNK], FP, space="PSUM", tag="o_ps")
                      nc.tensor.matmul(out=o_ps[:, 0:kn], lhsT=w4[:],
                                       rhs=h3_sb[:, 0:kn], start=True, stop=True)
                      o_sb = pool.tile([ncls, CHUNK], FP, tag="o_sb")
                      nc.vector.tensor_scalar(
                          out=o_sb[:, 0:kn], in0=o_ps[:, 0:kn], scalar1=b4[:, 0:1],
                          scalar2=None, op0=mybir.AluOpType.add)
                      nc.sync.dma_start(out=out_d[:, k0:k0 + kn], in_=o_sb[:, 0:kn])


            for _rep in range(reps):
                emit_pipeline()

    nc.compile()
    return nc


# ----------------------------------------------------------------------------
# entry point
# ----------------------------------------------------------------------------

_CACHE = {}


def _get_nc(dcfg):
    key = tuple(sorted(dcfg.items()))
    if key not in _CACHE:
        _CACHE[key] = build_device(dcfg)
    return _CACHE[key]


def kernel(x, edge_index, edge_attr,
           Wq1, bq1, Wk1, bk1, Wv1, bv1, We1, Ws1, bs1,
           Wq2, bq2, Wk2, bk2, Wv2, bv2, We2, Ws2, bs2,
           W3, b3, W4, b4):
    x = np.asarray(x, np.float32)
    n_nodes = x.shape[0]
    n_edges = np.asarray(edge_index).shape[1]
    percore, dcfg = host_prep(x, np.asarray(edge_index),
                              np.asarray(edge_attr, np.float32),
                              n_nodes, n_edges, np.asarray(edge_attr).shape[1])
    f32 = lambda a: np.ascontiguousarray(np.asarray(a, np.float32))
    weights = dict(
        wkv1=np.concatenate([f32(Wk1), f32(Wv1)], axis=1),
        bkv1=np.concatenate([f32(bk1), f32(bv1)])[None, :],
        wqs1=np.concatenate([f32(Wq1), f32(Ws1)], axis=1),
        bqs1=np.concatenate([f32(bq1), f32(bs1)])[None, :],
        we1=f32(We1),
        wkv2=np.concatenate([f32(Wk2), f32(Wv2)], axis=1),
        bkv2=np.concatenate([f32(bk2), f32(bv2)])[None, :],
        wqs2=np.concatenate([f32(Wq2), f32(Ws2)], axis=1),
        bqs2=np.concatenate([f32(bq2), f32(bs2)])[None, :],
        we2=f32(We2),
        w3=f32(W3), b3=f32(b3)[:, None],
        w4=f32(W4), b4=f32(b4)[:, None],
    )
    in_maps = [dict(pc, **weights) for pc in percore]
    nc = _get_nc(dcfg)
    res = run_bass_kernel_spmd(nc, in_maps, core_ids=list(range(NCORES)))
    out = np.concatenate([res.results[i]["out"].T for i in range(NCORES)])
    return np.ascontiguousarray(out[:n_nodes])

